# revision 1
# baseline (speedup 1.0000x reference)
"""MoE FFN (top-2 of 8 experts) on 8 Trainium2 NeuronCores.

Strategy (expert parallelism, per the sharding hint):
  - Host: router (softmax -> top-2 -> renorm) on [T, 8] logits — negligible
    FLOPs — then dispatch: gather each expert's tokens, transpose to [D, C]
    so the device needs no on-chip transposes at all.
  - Device (SPMD, one expert per core): hT = gelu(w1.T-accumulated matmul)
    with F on the partition axis (b1 becomes a per-partition activation
    bias), then y = hT.T @ w2 with hT used directly as the stationary
    operand, scaled by the per-token combine weight on the way out of PSUM.
    All matmuls bf16 with f32 PSUM accumulation.
  - Host: scatter-add the two expert contributions per token, plus the
    analytic sum_e cw[e,t]*b2[e] term.
"""

import os
import sys

sys.path.insert(0, "/opt/trn_rl_repo")

import numpy as np
import ml_dtypes

import concourse.bass as bass
import concourse.bacc as bacc
import concourse.mybir as mybir
from concourse import tile
from concourse.bass_utils import run_bass_kernel_spmd

BF16 = ml_dtypes.bfloat16
P = 128
D, F, E = 1024, 4096, 8
ND, NF = D // P, F // P  # 8, 32
TOP_K = 2

TRACE = bool(int(os.environ.get("MOE_TRACE", "0")))
TRACE_ALL = bool(int(os.environ.get("MOE_TRACE_ALL", "0")))
LAST = {}

_BUILD_CACHE = {}


def _enable_axon_profiling():
    """The image's antenv lacks axon_hooks, so boot() silently skipped NTFF
    hook registration. Recreate the module and register the ctypes hook so
    run_bass_kernel_spmd(trace=True) can profile. Also keep artifacts local."""
    import types
    import contextlib

    if "antenv.axon_hooks" not in sys.modules:
        mod = types.ModuleType("antenv.axon_hooks")
        mod._hook = None

        def set_axon_ntff_profile_hook(h):
            mod._hook = h

        def get_axon_ntff_profile_hook():
            return mod._hook

        mod.set_axon_ntff_profile_hook = set_axon_ntff_profile_hook
        mod.get_axon_ntff_profile_hook = get_axon_ntff_profile_hook
        sys.modules["antenv.axon_hooks"] = mod
        import antenv

        antenv.axon_hooks = mod
    hooks = sys.modules["antenv.axon_hooks"]
    if hooks.get_axon_ntff_profile_hook() is None:
        from trn_agent_boot.trn_boot import _ntff_profile_via_ctypes

        hooks.set_axon_ntff_profile_hook(
            _ntff_profile_via_ctypes("/opt/axon/libaxon_pjrt.so")
        )
    import concourse.bass_utils as bu

    bu.upload_artifacts = lambda tmpdir: tmpdir


if TRACE:
    _enable_axon_profiling()


CC = 512


def _chunks_for(C):
    ch = [CC] * (C // CC)
    if C % CC:
        ch.append(C % CC)
    return ch


def _build(C, act_func=None):
    """One expert's FFN over C (padded) tokens; SPMD across 8 cores."""
    if act_func is None:
        act_func = mybir.ActivationFunctionType.Gelu
    nc = bacc.Bacc()
    dt = mybir.dt
    xTc = nc.dram_tensor("xTc", [P, ND, C], dt.bfloat16, kind="ExternalInput")
    w1c = nc.dram_tensor("w1c", [P, NF // 4, ND, 512], dt.bfloat16, kind="ExternalInput")
    w2c = nc.dram_tensor("w2c", [P, NF, D], dt.bfloat16, kind="ExternalInput")
    b1c = nc.dram_tensor("b1c", [P, NF], dt.float32, kind="ExternalInput")
    cwc = nc.dram_tensor("cwc", [P, C // P], dt.float32, kind="ExternalInput")
    y = nc.dram_tensor("y", [C, D], dt.float32, kind="ExternalOutput")

    chunks = _chunks_for(C)
    with tile.TileContext(nc) as tc:
        with (
            tc.tile_pool(name="weights", bufs=1) as wpool,
            tc.tile_pool(name="consts", bufs=1) as cpool,
            tc.tile_pool(name="xin", bufs=2) as xpool,
            tc.tile_pool(name="hmid", bufs=1) as hpool,
            tc.tile_pool(name="yout", bufs=3) as ypool,
            tc.tile_pool(name="psh", bufs=4, space="PSUM") as psh,
            tc.tile_pool(name="psy", bufs=4, space="PSUM") as psy,
        ):
            # Weights as separate tiles per slice: dependency tracking is
            # tile-granular, so per-kd/per-group tiles let each matmul wait
            # only on its own slice's DMA instead of the full weight load.
            w1_sb = [wpool.tile([P, ND, 512], dt.bfloat16, name=f"w1_{g}", tag=f"w1_{g}") for g in range(NF // 4)]
            w2_sb = [wpool.tile([P, 4, D], dt.bfloat16, name=f"w2_{g}", tag=f"w2_{g}") for g in range(NF // 4)]
            b1_sb = cpool.tile([P, NF], dt.float32)
            cw_sb = cpool.tile([P, C // P], dt.float32)

            # chunk-0 activations first (small, on the critical path to the
            # very first matmul), then w1; w2 is issued just-in-time inside
            # chunk 0's matmul-1 loop so it doesn't steal load bandwidth.
            # per-kd tiles for chunk 0 so the first matmul only waits on the
            # first 128KB slice, not the whole chunk.
            xT0_sb = xpool.tile([P, ND, CC], dt.bfloat16, tag="xT")
            nc.sync.dma_start(
                out=xT0_sb[:, :, : chunks[0]], in_=xTc[:, :, : chunks[0]]
            )
            warm_l = cpool.tile([P, P], dt.bfloat16)
            nc.vector.memset(warm_l[:], 0.0)
            warm_ps = psy.tile([P, 512], dt.float32, tag="py")
            for i in range(20):
                nc.tensor.matmul(
                    warm_ps[:, :P], warm_l[:], warm_l[:],
                    start=(i == 0), stop=(i == 19),
                )

            for g in range(NF // 4):
                nc.sync.dma_start(out=w1_sb[g][:], in_=w1c[:, g, :, :])
            nc.sync.dma_start(out=b1_sb[:], in_=b1c[:])
            nc.sync.dma_start(out=cw_sb[:], in_=cwc[:])

            c0 = 0
            for ci, Cc in enumerate(chunks):
                ncb = Cc // P
                if ci == 0:
                    xT_sb = xT0_sb
                else:
                    xT_sb = xpool.tile([P, ND, CC], dt.bfloat16, tag="xT")
                    nc.sync.dma_start(
                        out=xT_sb[:, :, :Cc], in_=xTc[:, :, c0 : c0 + Cc]
                    )
                hT_sb = hpool.tile([P, NF, CC], dt.bfloat16, tag="hT")
                for fb in range(NF):
                    if ci == 0 and fb == 8:
                        for g in range(NF // 4):
                            nc.sync.dma_start(
                                out=w2_sb[g][:],
                                in_=w2c[:, g * 4 : (g + 1) * 4, :],
                            )
                    ph = psh.tile([P, CC], dt.float32, tag="ph")
                    for kd in range(ND):
                        nc.tensor.matmul(
                            ph[:, :Cc],
                            w1_sb[fb // 4][:, kd, (fb % 4) * P : (fb % 4 + 1) * P],
                            xT_sb[:, kd, :Cc],
                            start=(kd == 0),
                            stop=(kd == ND - 1),
                        )
                    nc.scalar.activation(
                        hT_sb[:, fb, :Cc],
                        ph[:, :Cc],
                        act_func,
                        bias=b1_sb[:, fb : fb + 1],
                    )
                for cb in range(ncb):
                    y_sb = ypool.tile([P, D], dt.float32, tag="y")
                    for dc in range(2):
                        py = psy.tile([P, 512], dt.float32, tag="py")
                        for fb in range(NF):
                            nc.tensor.matmul(
                                py[:],
                                hT_sb[:, fb, cb * P : (cb + 1) * P],
                                w2_sb[fb // 4][:, fb % 4, dc * 512 : (dc + 1) * 512],
                                start=(fb == 0),
                                stop=(fb == NF - 1),
                            )
                        blk = c0 // P + cb
                        nc.vector.tensor_scalar_mul(
                            y_sb[:, dc * 512 : (dc + 1) * 512],
                            py[:],
                            cw_sb[:, blk : blk + 1],
                        )
                        nc.sync.dma_start(
                            out=y[
                                c0 + cb * P : c0 + (cb + 1) * P,
                                dc * 512 : (dc + 1) * 512,
                            ],
                            in_=y_sb[:, dc * 512 : (dc + 1) * 512],
                        )
                c0 += Cc
    nc.compile()
    return nc


def _route(xf, router_w, router_b):
    """Replicates reference routing in numpy f32."""
    logits = xf @ router_w + router_b
    logits = logits - logits.max(axis=1, keepdims=True)
    p = np.exp(logits)
    p /= p.sum(axis=1, keepdims=True)
    top_i = np.argsort(-p, axis=1, kind="stable")[:, :TOP_K]
    tp = np.take_along_axis(p, top_i, 1)
    tp = tp / tp.sum(axis=1, keepdims=True)
    return top_i, tp.astype(np.float32)


def kernel(x, w1, b1, w2, b2, router_w, router_b):
    x = np.asarray(x, np.float32)
    B, S, _ = x.shape
    T = B * S
    xf = x.reshape(T, D)
    w1f = np.asarray(w1, np.float32)
    w2f = np.asarray(w2, np.float32)
    b1f = np.asarray(b1, np.float32)
    b2f = np.asarray(b2, np.float32)

    top_i, tp = _route(xf, np.asarray(router_w, np.float32), np.asarray(router_b, np.float32))

    idxs, cws = [], []
    for e in range(E):
        sel = top_i == e
        rows = np.nonzero(sel.any(axis=1))[0]
        w = (tp * sel).sum(axis=1)[rows]
        idxs.append(rows)
        cws.append(w.astype(np.float32))

    maxn = max(len(r) for r in idxs)
    C = max(CC, ((maxn + 127) // 128) * 128)

    if C not in _BUILD_CACHE:
        _BUILD_CACHE[C] = _build(C)
    nc = _BUILD_CACHE[C]

    w1b = w1f.astype(BF16)
    w2b = w2f.astype(BF16)
    in_maps = []
    for e in range(E):
        n = len(idxs[e])
        xT = np.zeros((P, ND, C), BF16)
        if n:
            g = xf[idxs[e]].astype(BF16).T  # [D, n]
            xT[:, :, :n] = g.reshape(ND, P, n).transpose(1, 0, 2)
        cwf = np.zeros(C, np.float32)
        cwf[:n] = cws[e]
        in_maps.append(
            {
                "xTc": xT,
                "w1c": np.ascontiguousarray(w1b[e].reshape(ND, P, NF // 4, 512).transpose(1, 2, 0, 3)),
                "w2c": np.ascontiguousarray(w2b[e].reshape(NF, P, D).transpose(1, 0, 2)),
                "b1c": np.ascontiguousarray(b1f[e].reshape(NF, P).T),
                "cwc": np.ascontiguousarray(cwf.reshape(C // P, P).T),
            }
        )

    res = run_bass_kernel_spmd(
        nc,
        in_maps,
        list(range(E)),
        trace=TRACE,
        trace_cores=list(range(E)) if TRACE_ALL else None,
    )
    LAST["exec_time_ns"] = res.exec_time_ns
    LAST["res"] = res
    LAST["C"] = C

    outf = np.zeros((T, D), np.float32)
    for e in range(E):
        n = len(idxs[e])
        if n:
            ye = np.asarray(res.results[e]["y"], np.float32)
            outf[idxs[e]] += ye[:n]
    # b2 enters as sum_e cw[e,t] * b2[e]
    cw_dense = np.zeros((T, E), np.float32)
    np.put_along_axis(cw_dense, top_i, tp, axis=1)
    outf += cw_dense @ b2f
    return outf.reshape(B, S, D)



# revision 2
# speedup vs baseline: 1.0301x; 1.0301x over previous
"""MoE FFN (top-2 of 8 experts) on 8 Trainium2 NeuronCores.

Strategy (expert parallelism, per the sharding hint):
  - Host: router (softmax -> top-2 -> renorm) on [T, 8] logits — negligible
    FLOPs — then dispatch: gather each expert's tokens, transpose to [D, C]
    so the device needs no on-chip transposes at all.
  - Device (SPMD, one expert per core): hT = gelu(w1.T-accumulated matmul)
    with F on the partition axis (b1 becomes a per-partition activation
    bias), then y = hT.T @ w2 with hT used directly as the stationary
    operand, scaled by the per-token combine weight on the way out of PSUM.
    All matmuls bf16 with f32 PSUM accumulation.
  - Host: scatter-add the two expert contributions per token, plus the
    analytic sum_e cw[e,t]*b2[e] term.

DMA plan (v2, from trace analysis of the 512us baseline):
  - Two parallel HWDGE rings: weights (b1, cw, w1, w2) on the Sync ring,
    activations in (xT chunks) and y out on the Scalar ring. b1/cw go FIRST
    (the baseline queued b1 behind 8MB of w1, stalling the PE 15us at fb=4
    when the first activation — which needs b1 — gated PSUM bank reuse).
  - w1 group 0 is split in two half tiles so the first matmul only waits on
    0.5MB; xT is stored chunk-contiguous in DRAM for full-rate transfers.
  - y is written bf16 (half the bytes; error budget allows), one DMA per
    128-token block.
"""

import os
import sys

sys.path.insert(0, "/opt/trn_rl_repo")

import numpy as np
import ml_dtypes

import concourse.bass as bass
import concourse.bacc as bacc
import concourse.mybir as mybir
from concourse import tile
from concourse.bass_utils import run_bass_kernel_spmd

BF16 = ml_dtypes.bfloat16
P = 128
D, F, E = 1024, 4096, 8
ND, NF = D // P, F // P  # 8, 32
TOP_K = 2

TRACE = bool(int(os.environ.get("MOE_TRACE", "0")))
TRACE_ALL = bool(int(os.environ.get("MOE_TRACE_ALL", "0")))
LAST = {}

_BUILD_CACHE = {}


def _enable_axon_profiling():
    """The image's antenv lacks axon_hooks, so boot() silently skipped NTFF
    hook registration. Recreate the module and register the ctypes hook so
    run_bass_kernel_spmd(trace=True) can profile. Also keep artifacts local."""
    import types
    import contextlib

    if "antenv.axon_hooks" not in sys.modules:
        mod = types.ModuleType("antenv.axon_hooks")
        mod._hook = None

        def set_axon_ntff_profile_hook(h):
            mod._hook = h

        def get_axon_ntff_profile_hook():
            return mod._hook

        mod.set_axon_ntff_profile_hook = set_axon_ntff_profile_hook
        mod.get_axon_ntff_profile_hook = get_axon_ntff_profile_hook
        sys.modules["antenv.axon_hooks"] = mod
        import antenv

        antenv.axon_hooks = mod
    hooks = sys.modules["antenv.axon_hooks"]
    if hooks.get_axon_ntff_profile_hook() is None:
        from trn_agent_boot.trn_boot import _ntff_profile_via_ctypes

        hooks.set_axon_ntff_profile_hook(
            _ntff_profile_via_ctypes("/opt/axon/libaxon_pjrt.so")
        )
    import concourse.bass_utils as bu

    bu.upload_artifacts = lambda tmpdir: tmpdir


if TRACE:
    _enable_axon_profiling()


CC = 512
WARMUP_MM = 36


def _chunks_for(C):
    ch = [CC] * (C // CC)
    if C % CC:
        ch.append(C % CC)
    return ch


def _build(C, act_func=None):
    """One expert's FFN over C (padded) tokens; SPMD across 8 cores."""
    if act_func is None:
        act_func = mybir.ActivationFunctionType.Gelu
    nc = bacc.Bacc()
    dt = mybir.dt
    chunks = _chunks_for(C)
    NCH = len(chunks)
    xTc = nc.dram_tensor("xTc", [P, NCH, ND, CC], dt.bfloat16, kind="ExternalInput")
    w1c = nc.dram_tensor("w1c", [P, NF // 4, ND, 512], dt.bfloat16, kind="ExternalInput")
    w2c = nc.dram_tensor("w2c", [P, NF, D], dt.bfloat16, kind="ExternalInput")
    b1c = nc.dram_tensor("b1c", [P, NF], dt.float32, kind="ExternalInput")
    cwc = nc.dram_tensor("cwc", [P, C // P], dt.float32, kind="ExternalInput")
    y = nc.dram_tensor("y", [C, D], dt.bfloat16, kind="ExternalOutput")

    with tile.TileContext(nc) as tc:
        with (
            tc.tile_pool(name="weights", bufs=1) as wpool,
            tc.tile_pool(name="consts", bufs=1) as cpool,
            tc.tile_pool(name="xin", bufs=4) as xpool,
            tc.tile_pool(name="hmid", bufs=1) as hpool,
            tc.tile_pool(name="yout", bufs=3) as ypool,
            tc.tile_pool(name="psh", bufs=4, space="PSUM") as psh,
            tc.tile_pool(name="psy", bufs=4, space="PSUM") as psy,
        ):
            # Weights as separate tiles per slice: dependency tracking is
            # tile-granular, so per-group tiles let each matmul wait only on
            # its own slice's DMA instead of the full weight load. Group 0 is
            # further halved so the very first matmul waits on just 0.5MB.
            w1_g0 = [wpool.tile([P, 4, 512], dt.bfloat16, name=f"w1_0{h}", tag=f"w1_0{h}") for h in range(2)]
            w1_sb = [wpool.tile([P, ND, 512], dt.bfloat16, name=f"w1_{g}", tag=f"w1_{g}") for g in range(1, NF // 4)]
            w2_sb = [wpool.tile([P, 4, D], dt.bfloat16, name=f"w2_{g}", tag=f"w2_{g}") for g in range(NF // 4)]
            b1_sb = cpool.tile([P, NF], dt.float32)
            cw_sb = cpool.tile([P, C // P], dt.float32)

            def w1_slice(fb, kd):
                g = fb // 4
                if g == 0:
                    return w1_g0[kd // 4][:, kd % 4, (fb % 4) * P : (fb % 4 + 1) * P]
                return w1_sb[g - 1][:, kd, (fb % 4) * P : (fb % 4 + 1) * P]

            # Scalar ring: first four x chunks (in parallel with the weight
            # stream on the Sync ring), later the y output DMAs.
            xT_sb = []
            for c in range(NCH):
                t = xpool.tile([P, ND, CC], dt.bfloat16, tag="xT")
                xT_sb.append(t)
                if c < 4:
                    nc.scalar.dma_start(out=t[:], in_=xTc[:, c])

            # Sync ring: tiny consts first (b1 gates the first activation and
            # with it PSUM bank reuse), then w1, then w2, then any leftover x
            # chunks (their pool-reuse waits would block a ring, so they go
            # dead last on the weights ring after everything that matters).
            nc.sync.dma_start(out=b1_sb[:], in_=b1c[:])
            nc.sync.dma_start(out=cw_sb[:], in_=cwc[:])
            nc.sync.dma_start(out=w1_g0[0][:], in_=w1c[:, 0, 0:4, :])
            nc.sync.dma_start(out=w1_g0[1][:], in_=w1c[:, 0, 4:8, :])
            for g in range(1, NF // 4):
                nc.sync.dma_start(out=w1_sb[g - 1][:], in_=w1c[:, g, :, :])
            for g in range(NF // 4):
                nc.sync.dma_start(out=w2_sb[g][:], in_=w2c[:, g * 4 : (g + 1) * 4, :])
            for c in range(4, NCH):
                nc.sync.dma_start(out=xT_sb[c][:], in_=xTc[:, c])

            # Tensor-engine warmup: dummy matmuls ramp the PE p-state (full
            # clock needs ~3us of continuous execution) while the first
            # transfers land; sized to end roughly when w1 g0 arrives.
            warm_l = cpool.tile([P, P], dt.bfloat16)
            nc.vector.memset(warm_l[:], 0.0)
            warm_ps = psy.tile([P, 512], dt.float32, tag="py")
            for i in range(WARMUP_MM):
                nc.tensor.matmul(
                    warm_ps[:, :P], warm_l[:], warm_l[:],
                    start=(i == 0), stop=(i == WARMUP_MM - 1),
                )

            c0 = 0
            for ci, Cc in enumerate(chunks):
                ncb = Cc // P
                hT_sb = hpool.tile([P, NF, CC], dt.bfloat16, tag="hT")
                for fb in range(NF):
                    ph = psh.tile([P, CC], dt.float32, tag="ph")
                    for kd in range(ND):
                        nc.tensor.matmul(
                            ph[:, :Cc],
                            w1_slice(fb, kd),
                            xT_sb[ci][:, kd, :Cc],
                            start=(kd == 0),
                            stop=(kd == ND - 1),
                        )
                    nc.scalar.activation(
                        hT_sb[:, fb, :Cc],
                        ph[:, :Cc],
                        act_func,
                        bias=b1_sb[:, fb : fb + 1],
                    )
                for cb in range(ncb):
                    y_sb = ypool.tile([P, D], dt.bfloat16, tag="y")
                    blk = c0 // P + cb
                    for dc in range(2):
                        py = psy.tile([P, 512], dt.float32, tag="py")
                        for fb in range(NF):
                            nc.tensor.matmul(
                                py[:],
                                hT_sb[:, fb, cb * P : (cb + 1) * P],
                                w2_sb[fb // 4][:, fb % 4, dc * 512 : (dc + 1) * 512],
                                start=(fb == 0),
                                stop=(fb == NF - 1),
                            )
                        nc.vector.tensor_scalar_mul(
                            y_sb[:, dc * 512 : (dc + 1) * 512],
                            py[:],
                            cw_sb[:, blk : blk + 1],
                        )
                    nc.scalar.dma_start(
                        out=y[c0 + cb * P : c0 + (cb + 1) * P, :],
                        in_=y_sb[:],
                    )
                c0 += Cc
    nc.compile()
    return nc


def _route(xf, router_w, router_b):
    """Replicates reference routing in numpy f32."""
    logits = xf @ router_w + router_b
    logits = logits - logits.max(axis=1, keepdims=True)
    p = np.exp(logits)
    p /= p.sum(axis=1, keepdims=True)
    top_i = np.argsort(-p, axis=1, kind="stable")[:, :TOP_K]
    tp = np.take_along_axis(p, top_i, 1)
    tp = tp / tp.sum(axis=1, keepdims=True)
    return top_i, tp.astype(np.float32)


def kernel(x, w1, b1, w2, b2, router_w, router_b):
    x = np.asarray(x, np.float32)
    B, S, _ = x.shape
    T = B * S
    xf = x.reshape(T, D)
    w1f = np.asarray(w1, np.float32)
    w2f = np.asarray(w2, np.float32)
    b1f = np.asarray(b1, np.float32)
    b2f = np.asarray(b2, np.float32)

    top_i, tp = _route(xf, np.asarray(router_w, np.float32), np.asarray(router_b, np.float32))

    idxs, cws = [], []
    for e in range(E):
        sel = top_i == e
        rows = np.nonzero(sel.any(axis=1))[0]
        w = (tp * sel).sum(axis=1)[rows]
        idxs.append(rows)
        cws.append(w.astype(np.float32))

    maxn = max(len(r) for r in idxs)
    C = max(CC, ((maxn + 127) // 128) * 128)

    if C not in _BUILD_CACHE:
        _BUILD_CACHE[C] = _build(C)
    nc = _BUILD_CACHE[C]

    chunks = _chunks_for(C)
    NCH = len(chunks)
    Cpad = NCH * CC

    w1b = w1f.astype(BF16)
    w2b = w2f.astype(BF16)
    in_maps = []
    for e in range(E):
        n = len(idxs[e])
        xT = np.zeros((P, ND, Cpad), BF16)
        if n:
            g = xf[idxs[e]].astype(BF16).T  # [D, n]
            xT[:, :, :n] = g.reshape(ND, P, n).transpose(1, 0, 2)
        # chunk-contiguous layout: [P, NCH, ND, CC]
        xT = np.ascontiguousarray(xT.reshape(P, ND, NCH, CC).transpose(0, 2, 1, 3))
        cwf = np.zeros(C, np.float32)
        cwf[:n] = cws[e]
        in_maps.append(
            {
                "xTc": xT,
                "w1c": np.ascontiguousarray(w1b[e].reshape(ND, P, NF // 4, 512).transpose(1, 2, 0, 3)),
                "w2c": np.ascontiguousarray(w2b[e].reshape(NF, P, D).transpose(1, 0, 2)),
                "b1c": np.ascontiguousarray(b1f[e].reshape(NF, P).T),
                "cwc": np.ascontiguousarray(cwf.reshape(C // P, P).T),
            }
        )

    res = run_bass_kernel_spmd(
        nc,
        in_maps,
        list(range(E)),
        trace=TRACE,
        trace_cores=list(range(E)) if TRACE_ALL else None,
    )
    LAST["exec_time_ns"] = res.exec_time_ns
    LAST["res"] = res
    LAST["C"] = C

    outf = np.zeros((T, D), np.float32)
    for e in range(E):
        n = len(idxs[e])
        if n:
            ye = np.asarray(res.results[e]["y"][:n], np.float32)
            outf[idxs[e]] += ye
    # b2 enters as sum_e cw[e,t] * b2[e]
    cw_dense = np.zeros((T, E), np.float32)
    np.put_along_axis(cw_dense, top_i, tp, axis=1)
    outf += cw_dense @ b2f
    return outf.reshape(B, S, D)


# revision 5
# speedup vs baseline: 1.0680x; 1.0368x over previous
"""MoE FFN (top-2 of 8 experts) on 8 Trainium2 NeuronCores.

Strategy (expert parallelism, per the sharding hint):
  - Host: router (softmax -> top-2 -> renorm) on [T, 8] logits — negligible
    FLOPs — then dispatch: gather each expert's tokens, transpose to [D, C]
    so the device needs no on-chip transposes at all.
  - Device (SPMD, one expert per core): hT = gelu(w1.T-accumulated matmul)
    with F on the partition axis (b1 becomes a per-partition activation
    bias), then y = hT.T @ w2 with hT used directly as the stationary
    operand, scaled by the per-token combine weight on the way out of PSUM.
    All matmuls bf16 with f32 PSUM accumulation.
  - Host: scatter-add the two expert contributions per token, plus the
    analytic sum_e cw[e,t]*b2[e] term.

DMA plan (v2, from trace analysis of the 512us baseline):
  - Two parallel HWDGE rings: weights (b1, cw, w1, w2) on the Sync ring,
    activations in (xT chunks) and y out on the Scalar ring. b1/cw go FIRST
    (the baseline queued b1 behind 8MB of w1, stalling the PE 15us at fb=4
    when the first activation — which needs b1 — gated PSUM bank reuse).
  - w1 group 0 is split in two half tiles so the first matmul only waits on
    0.5MB; xT is stored chunk-contiguous in DRAM for full-rate transfers.
  - y is written bf16 (half the bytes; error budget allows), one DMA per
    128-token block.
"""

import os
import sys

sys.path.insert(0, "/opt/trn_rl_repo")

import numpy as np
import ml_dtypes

import concourse.bass as bass
import concourse.bacc as bacc
import concourse.mybir as mybir
from concourse import tile
from concourse.bass_utils import run_bass_kernel_spmd

BF16 = ml_dtypes.bfloat16
P = 128
D, F, E = 1024, 4096, 8
ND, NF = D // P, F // P  # 8, 32
TOP_K = 2

TRACE = bool(int(os.environ.get("MOE_TRACE", "0")))
TRACE_ALL = bool(int(os.environ.get("MOE_TRACE_ALL", "0")))
LAST = {}

_BUILD_CACHE = {}


def _enable_axon_profiling():
    """The image's antenv lacks axon_hooks, so boot() silently skipped NTFF
    hook registration. Recreate the module and register the ctypes hook so
    run_bass_kernel_spmd(trace=True) can profile. Also keep artifacts local."""
    import types
    import contextlib

    if "antenv.axon_hooks" not in sys.modules:
        mod = types.ModuleType("antenv.axon_hooks")
        mod._hook = None

        def set_axon_ntff_profile_hook(h):
            mod._hook = h

        def get_axon_ntff_profile_hook():
            return mod._hook

        mod.set_axon_ntff_profile_hook = set_axon_ntff_profile_hook
        mod.get_axon_ntff_profile_hook = get_axon_ntff_profile_hook
        sys.modules["antenv.axon_hooks"] = mod
        import antenv

        antenv.axon_hooks = mod
    hooks = sys.modules["antenv.axon_hooks"]
    if hooks.get_axon_ntff_profile_hook() is None:
        from trn_agent_boot.trn_boot import _ntff_profile_via_ctypes

        hooks.set_axon_ntff_profile_hook(
            _ntff_profile_via_ctypes("/opt/axon/libaxon_pjrt.so")
        )
    import concourse.bass_utils as bu

    bu.upload_artifacts = lambda tmpdir: tmpdir


if TRACE:
    _enable_axon_profiling()


CC = 512
WARMUP_MM = 36


def _chunks_for(C):
    ch = [CC] * (C // CC)
    if C % CC:
        ch.append(C % CC)
    return ch


def _build(C, act_func=None):
    """One expert's FFN over C (padded) tokens; SPMD across 8 cores."""
    if act_func is None:
        act_func = mybir.ActivationFunctionType.Gelu
    nc = bacc.Bacc()
    dt = mybir.dt
    chunks = _chunks_for(C)
    NCH = len(chunks)
    xTc = nc.dram_tensor("xTc", [P, NCH, ND, CC], dt.bfloat16, kind="ExternalInput")
    w1c = nc.dram_tensor("w1c", [P, NF // 4, ND, 512], dt.bfloat16, kind="ExternalInput")
    w2c = nc.dram_tensor("w2c", [P, NF, D], dt.bfloat16, kind="ExternalInput")
    b1c = nc.dram_tensor("b1c", [P, NF], dt.float32, kind="ExternalInput")
    cwc = nc.dram_tensor("cwc", [P, C // P], dt.float32, kind="ExternalInput")
    y = nc.dram_tensor("y", [C, D], dt.bfloat16, kind="ExternalOutput")

    with tile.TileContext(nc) as tc:
        with (
            tc.tile_pool(name="weights", bufs=1) as wpool,
            tc.tile_pool(name="consts", bufs=1) as cpool,
            tc.tile_pool(name="xin", bufs=4) as xpool,
            tc.tile_pool(name="hmid", bufs=1) as hpool,
            tc.tile_pool(name="yout", bufs=3) as ypool,
            tc.tile_pool(name="psh", bufs=4, space="PSUM") as psh,
            tc.tile_pool(name="psy", bufs=4, space="PSUM") as psy,
        ):
            # Weights as separate tiles per slice: dependency tracking is
            # tile-granular, so per-group tiles let each matmul wait only on
            # its own slice's DMA instead of the full weight load. Group 0 is
            # further halved so the very first matmul waits on just 0.5MB.
            w1_g0 = [wpool.tile([P, 4, 512], dt.bfloat16, name=f"w1_0{h}", tag=f"w1_0{h}") for h in range(2)]
            w1_sb = [wpool.tile([P, ND, 512], dt.bfloat16, name=f"w1_{g}", tag=f"w1_{g}") for g in range(1, NF // 4)]
            w2_sb = [wpool.tile([P, 4, D], dt.bfloat16, name=f"w2_{g}", tag=f"w2_{g}") for g in range(NF // 4)]
            b1_sb = cpool.tile([P, NF], dt.float32)
            cw_sb = cpool.tile([P, C // P], dt.float32)

            def w1_slice(fb, kd):
                g = fb // 4
                if g == 0:
                    return w1_g0[kd // 4][:, kd % 4, (fb % 4) * P : (fb % 4 + 1) * P]
                return w1_sb[g - 1][:, kd, (fb % 4) * P : (fb % 4 + 1) * P]

            # Scalar ring: chunk 0 now; chunks 1-3 are issued from inside
            # chunk 0's activation stream so their transfers don't steal
            # SDMA bandwidth from the startup-critical w1 stream.
            xT_sb = []
            for c in range(NCH):
                t = xpool.tile([P, ND, CC], dt.bfloat16, tag="xT")
                xT_sb.append(t)
                if c < 1:
                    nc.scalar.dma_start(out=t[:], in_=xTc[:, c])

            # Sync ring: tiny consts first (b1 gates the first activation and
            # with it PSUM bank reuse), then w1, then w2, then any leftover x
            # chunks (their pool-reuse waits would block a ring, so they go
            # dead last on the weights ring after everything that matters).
            nc.sync.dma_start(out=b1_sb[:], in_=b1c[:])
            nc.sync.dma_start(out=cw_sb[:], in_=cwc[:])
            nc.sync.dma_start(out=w1_g0[0][:], in_=w1c[:, 0, 0:4, :])
            nc.sync.dma_start(out=w1_g0[1][:], in_=w1c[:, 0, 4:8, :])
            for g in range(1, NF // 4):
                nc.sync.dma_start(out=w1_sb[g - 1][:], in_=w1c[:, g, :, :])
            for g in range(NF // 4):
                nc.sync.dma_start(out=w2_sb[g][:], in_=w2c[:, g * 4 : (g + 1) * 4, :])
            for c in range(4, NCH):
                nc.sync.dma_start(out=xT_sb[c][:], in_=xTc[:, c])

            # Tensor-engine warmup: dummy matmuls ramp the PE p-state (full
            # clock needs ~3us of continuous execution) while the first
            # transfers land; sized to end roughly when w1 g0 arrives.
            warm_l = cpool.tile([P, P], dt.bfloat16)
            nc.vector.memset(warm_l[:], 0.0)
            warm_ps = psy.tile([P, 512], dt.float32, tag="py")
            for i in range(WARMUP_MM):
                nc.tensor.matmul(
                    warm_ps[:, :P], warm_l[:], warm_l[:],
                    start=(i == 0), stop=(i == WARMUP_MM - 1),
                )

            c0 = 0
            for ci, Cc in enumerate(chunks):
                ncb = Cc // P
                hT_sb = hpool.tile([P, NF, CC], dt.bfloat16, tag="hT")
                for fb in range(NF):
                    ph = psh.tile([P, CC], dt.float32, tag="ph")
                    for kd in range(ND):
                        nc.tensor.matmul(
                            ph[:, :Cc],
                            w1_slice(fb, kd),
                            xT_sb[ci][:, kd, :Cc],
                            start=(kd == 0),
                            stop=(kd == ND - 1),
                        )
                    nc.scalar.activation(
                        hT_sb[:, fb, :Cc],
                        ph[:, :Cc],
                        act_func,
                        bias=b1_sb[:, fb : fb + 1],
                    )
                    if ci == 0 and 1 <= fb <= 3 and fb < NCH:
                        nc.scalar.dma_start(
                            out=xT_sb[fb][:], in_=xTc[:, fb]
                        )
                for cb in range(ncb):
                    y_sb = ypool.tile([P, D], dt.bfloat16, tag="y")
                    blk = c0 // P + cb
                    for dc in range(2):
                        py = psy.tile([P, 512], dt.float32, tag="py")
                        for fb in range(NF):
                            nc.tensor.matmul(
                                py[:],
                                hT_sb[:, fb, cb * P : (cb + 1) * P],
                                w2_sb[fb // 4][:, fb % 4, dc * 512 : (dc + 1) * 512],
                                start=(fb == 0),
                                stop=(fb == NF - 1),
                            )
                        nc.vector.tensor_scalar_mul(
                            y_sb[:, dc * 512 : (dc + 1) * 512],
                            py[:],
                            cw_sb[:, blk : blk + 1],
                        )
                        # per-half DMA so the final transfer off the last
                        # PSUM tile is half as long on the critical tail
                        nc.scalar.dma_start(
                            out=y[
                                c0 + cb * P : c0 + (cb + 1) * P,
                                dc * 512 : (dc + 1) * 512,
                            ],
                            in_=y_sb[:, dc * 512 : (dc + 1) * 512],
                        )
                c0 += Cc
    nc.compile()
    return nc


def _route(xf, router_w, router_b):
    """Replicates reference routing in numpy f32."""
    logits = xf @ router_w + router_b
    logits = logits - logits.max(axis=1, keepdims=True)
    p = np.exp(logits)
    p /= p.sum(axis=1, keepdims=True)
    top_i = np.argsort(-p, axis=1, kind="stable")[:, :TOP_K]
    tp = np.take_along_axis(p, top_i, 1)
    tp = tp / tp.sum(axis=1, keepdims=True)
    return top_i, tp.astype(np.float32)


def kernel(x, w1, b1, w2, b2, router_w, router_b):
    x = np.asarray(x, np.float32)
    B, S, _ = x.shape
    T = B * S
    xf = x.reshape(T, D)
    w1f = np.asarray(w1, np.float32)
    w2f = np.asarray(w2, np.float32)
    b1f = np.asarray(b1, np.float32)
    b2f = np.asarray(b2, np.float32)

    top_i, tp = _route(xf, np.asarray(router_w, np.float32), np.asarray(router_b, np.float32))

    idxs, cws = [], []
    for e in range(E):
        sel = top_i == e
        rows = np.nonzero(sel.any(axis=1))[0]
        w = (tp * sel).sum(axis=1)[rows]
        idxs.append(rows)
        cws.append(w.astype(np.float32))

    maxn = max(len(r) for r in idxs)
    C = max(CC, ((maxn + 127) // 128) * 128)

    if C not in _BUILD_CACHE:
        _BUILD_CACHE[C] = _build(C)
    nc = _BUILD_CACHE[C]

    chunks = _chunks_for(C)
    NCH = len(chunks)
    Cpad = NCH * CC

    w1b = w1f.astype(BF16)
    w2b = w2f.astype(BF16)
    in_maps = []
    for e in range(E):
        n = len(idxs[e])
        xT = np.zeros((P, ND, Cpad), BF16)
        if n:
            g = xf[idxs[e]].astype(BF16).T  # [D, n]
            xT[:, :, :n] = g.reshape(ND, P, n).transpose(1, 0, 2)
        # chunk-contiguous layout: [P, NCH, ND, CC]
        xT = np.ascontiguousarray(xT.reshape(P, ND, NCH, CC).transpose(0, 2, 1, 3))
        cwf = np.zeros(C, np.float32)
        cwf[:n] = cws[e]
        in_maps.append(
            {
                "xTc": xT,
                "w1c": np.ascontiguousarray(w1b[e].reshape(ND, P, NF // 4, 512).transpose(1, 2, 0, 3)),
                "w2c": np.ascontiguousarray(w2b[e].reshape(NF, P, D).transpose(1, 0, 2)),
                "b1c": np.ascontiguousarray(b1f[e].reshape(NF, P).T),
                "cwc": np.ascontiguousarray(cwf.reshape(C // P, P).T),
            }
        )

    res = run_bass_kernel_spmd(
        nc,
        in_maps,
        list(range(E)),
        trace=TRACE,
        trace_cores=list(range(E)) if TRACE_ALL else None,
    )
    LAST["exec_time_ns"] = res.exec_time_ns
    LAST["res"] = res
    LAST["C"] = C

    outf = np.zeros((T, D), np.float32)
    for e in range(E):
        n = len(idxs[e])
        if n:
            ye = np.asarray(res.results[e]["y"][:n], np.float32)
            outf[idxs[e]] += ye
    # b2 enters as sum_e cw[e,t] * b2[e]
    cw_dense = np.zeros((T, E), np.float32)
    np.put_along_axis(cw_dense, top_i, tp, axis=1)
    outf += cw_dense @ b2f
    return outf.reshape(B, S, D)


# revision 6
# speedup vs baseline: 1.0801x; 1.0114x over previous
"""MoE FFN (top-2 of 8 experts) on 8 Trainium2 NeuronCores — v4.

Sharding: hidden-dimension (F) slicing instead of expert parallelism.
Every core processes ALL experts' routed tokens, but only a 512-wide slice
of the 4096 hidden units (w1[:, :, fsl], w2[:, fsl, :]). Host sums the 8
partial y contributions. This removes the max-expert-load imbalance that
expert parallelism pays under SPMD: per-core work is sum_e pad128(n_e)/8
token-blocks instead of max_e pad128(n_e), a ~3.7% FLOP reduction here.

Per core, for each expert e and each 512-token chunk of its tokens:
  hT = gelu(w1[e][:, fsl].T @ xT + b1[e][fsl])   # [512f, tok] in 4 fb tiles
  y_part = cw * (hT.T @ w2[e][fsl, :])           # [tok, 1024] bf16 out

DMA: weights + y output on the Sync ring; xT chunks (34MB, identical array
for every core) stream on the Scalar ring, triggers interleaved two chunks
ahead of consumption.
"""

import os
import sys

sys.path.insert(0, "/opt/trn_rl_repo")

import numpy as np
import ml_dtypes

import concourse.bass as bass
import concourse.bacc as bacc
import concourse.mybir as mybir
from concourse import tile
from concourse.bass_utils import run_bass_kernel_spmd

BF16 = ml_dtypes.bfloat16
P = 128
D, F, E = 1024, 4096, 8
ND = D // P  # 8
NFC = 4      # fb blocks per core (F/8 = 512 = 4 * 128)
TOP_K = 2

TRACE = bool(int(os.environ.get("MOE_TRACE", "0")))
TRACE_ALL = bool(int(os.environ.get("MOE_TRACE_ALL", "0")))
LAST = {}

_BUILD_CACHE = {}


def _enable_axon_profiling():
    import types

    if "antenv.axon_hooks" not in sys.modules:
        mod = types.ModuleType("antenv.axon_hooks")
        mod._hook = None

        def set_axon_ntff_profile_hook(h):
            mod._hook = h

        def get_axon_ntff_profile_hook():
            return mod._hook

        mod.set_axon_ntff_profile_hook = set_axon_ntff_profile_hook
        mod.get_axon_ntff_profile_hook = get_axon_ntff_profile_hook
        sys.modules["antenv.axon_hooks"] = mod
        import antenv

        antenv.axon_hooks = mod
    hooks = sys.modules["antenv.axon_hooks"]
    if hooks.get_axon_ntff_profile_hook() is None:
        from trn_agent_boot.trn_boot import _ntff_profile_via_ctypes

        hooks.set_axon_ntff_profile_hook(
            _ntff_profile_via_ctypes("/opt/axon/libaxon_pjrt.so")
        )
    import concourse.bass_utils as bu

    bu.upload_artifacts = lambda tmpdir: tmpdir


if TRACE:
    _enable_axon_profiling()


CC = 512
WARMUP_MM = 48


def _chunks_for(C):
    ch = [CC] * (C // CC)
    if C % CC:
        ch.append(C % CC)
    return ch


def _build(caps):
    """caps: tuple of per-expert padded capacities (multiples of 128)."""
    act_func = mybir.ActivationFunctionType.Gelu
    nc = bacc.Bacc()
    dt = mybir.dt

    # global chunk schedule: (expert, chunk_size, token_offset_in_expert).
    # The very first chunk is split to 256 tokens so the startup-critical
    # xT transfer is half as long.
    sched = []
    for e, C_e in enumerate(caps):
        off = 0
        if e == 0 and C_e >= CC:
            ch = [CC // 4, 3 * CC // 4] + _chunks_for(C_e - CC)
        else:
            ch = _chunks_for(C_e)
        for Cc in ch:
            sched.append((e, Cc, off))
            off += Cc
    NCH = len(sched)
    NBT = sum(caps) // P  # total 128-token blocks

    # flat variable-width xT: chunk ci occupies columns [xoff[ci], xoff[ci+1])
    xoff = [0]
    for (_, Cc, _) in sched:
        xoff.append(xoff[-1] + ND * Cc)
    XW = xoff[-1]

    xTe = nc.dram_tensor("xTe", [P, XW], dt.bfloat16, kind="ExternalInput")
    w1s = nc.dram_tensor("w1s", [P, E, ND, 512], dt.bfloat16, kind="ExternalInput")
    w2s = nc.dram_tensor("w2s", [P, E, NFC, D], dt.bfloat16, kind="ExternalInput")
    b1s = nc.dram_tensor("b1s", [P, E, NFC], dt.float32, kind="ExternalInput")
    cws = nc.dram_tensor("cws", [P, NBT], dt.float32, kind="ExternalInput")
    y = nc.dram_tensor("y", [NBT * P, D], dt.bfloat16, kind="ExternalOutput")

    with tile.TileContext(nc) as tc:
        with (
            tc.tile_pool(name="weights", bufs=1) as wpool,
            tc.tile_pool(name="consts", bufs=1) as cpool,
            tc.tile_pool(name="xin", bufs=5) as xpool,
            tc.tile_pool(name="hmid", bufs=2) as hpool,
            tc.tile_pool(name="yout", bufs=6) as ypool,
            tc.tile_pool(name="psh", bufs=4, space="PSUM") as psh,
            tc.tile_pool(name="psy", bufs=4, space="PSUM") as psy,
        ):
            # per-expert weight tiles; expert 0's w1 halved so the first
            # matmul waits on only 0.5MB
            w1_e0 = [wpool.tile([P, 4, 512], dt.bfloat16, name=f"w1_0{h}", tag=f"w1_0{h}") for h in range(2)]
            w1_sb = [wpool.tile([P, ND, 512], dt.bfloat16, name=f"w1_{e}", tag=f"w1_{e}") for e in range(1, E)]
            w2_sb = [wpool.tile([P, NFC, D], dt.bfloat16, name=f"w2_{e}", tag=f"w2_{e}") for e in range(E)]
            b1_sb = cpool.tile([P, E, NFC], dt.float32)
            cw_sb = cpool.tile([P, NBT], dt.float32)

            def w1_slice(e, fb, kd):
                if e == 0:
                    return w1_e0[kd // 4][:, kd % 4, fb * P : (fb + 1) * P]
                return w1_sb[e - 1][:, kd, fb * P : (fb + 1) * P]

            xT_sb = [
                xpool.tile([P, ND * sched[c][1]], dt.bfloat16, name=f"xT_{c}", tag="xT")
                for c in range(NCH)
            ]

            def issue_x(c):
                nc.scalar.dma_start(
                    out=xT_sb[c][:], in_=xTe[:, xoff[c] : xoff[c + 1]]
                )

            # chunks 0-1 now; later chunks are triggered from inside the
            # activation stream two chunks ahead
            issue_x(0)
            issue_x(1)

            # Upfront only what the first two experts need (~4MB), ordered
            # by need time — the rest of w1/w2 is issued just-in-time from
            # inside the compute stream so the y output DMAs (same Sync
            # ring, strict FIFO) never queue behind bulk weight traffic.
            nc.sync.dma_start(out=w1_e0[0][:], in_=w1s[:, 0, 0:4, :])
            nc.sync.dma_start(out=w1_e0[1][:], in_=w1s[:, 0, 4:8, :])
            nc.sync.dma_start(out=b1_sb[:], in_=b1s[:])
            nc.sync.dma_start(out=w1_sb[0][:], in_=w1s[:, 1])
            nc.sync.dma_start(out=w2_sb[0][:], in_=w2s[:, 0])
            nc.sync.dma_start(out=cw_sb[:], in_=cws[:])
            nc.sync.dma_start(out=w2_sb[1][:], in_=w2s[:, 1])

            def issue_expert_weights(e):
                if e < E:
                    nc.sync.dma_start(out=w1_sb[e - 1][:], in_=w1s[:, e])
                    nc.sync.dma_start(out=w2_sb[e][:], in_=w2s[:, e])

            warm_l = cpool.tile([P, P], dt.bfloat16)
            nc.vector.memset(warm_l[:], 0.0)
            warm_ps = psy.tile([P, 512], dt.float32, tag="py")
            for i in range(WARMUP_MM):
                nc.tensor.matmul(
                    warm_ps[:, :P], warm_l[:], warm_l[:],
                    start=(i == 0), stop=(i == WARMUP_MM - 1),
                )

            blk0 = 0
            issued = 1
            first_chunk_of = {}
            for ci, (e, Cc, off) in enumerate(sched):
                first_chunk_of.setdefault(e, ci)
            for ci, (e, Cc, off) in enumerate(sched):
                ncb = Cc // P
                hT_sb = hpool.tile([P, NFC, CC], dt.bfloat16, tag="hT")
                for fb in range(NFC):
                    ph = psh.tile([P, CC], dt.float32, tag="ph")
                    for kd in range(ND):
                        nc.tensor.matmul(
                            ph[:, :Cc],
                            w1_slice(e, fb, kd),
                            xT_sb[ci][:, kd * Cc : (kd + 1) * Cc],
                            start=(kd == 0),
                            stop=(kd == ND - 1),
                        )
                    nc.scalar.activation(
                        hT_sb[:, fb, :Cc],
                        ph[:, :Cc],
                        act_func,
                        bias=b1_sb[:, e, fb : fb + 1],
                    )
                    # stream the x chunks two ahead of consumption; trigger
                    # only after the last activation of the chunk so the
                    # trigger's engine time never delays an activation that
                    # gates matmul-2's short accumulation chains
                    if fb == NFC - 1:
                        while issued < min(ci + 2, NCH - 1):
                            issued += 1
                            issue_x(issued)
                        if first_chunk_of.get(e) == ci:
                            issue_expert_weights(e + 2)
                for cb in range(ncb):
                    y_sb = ypool.tile([P, D], dt.bfloat16, tag="y")
                    blk = blk0 + cb
                    last_blk = ci == NCH - 1 and cb == ncb - 1
                    for dc in range(2):
                        py = psy.tile([P, 512], dt.float32, tag="py")
                        for fb in range(NFC):
                            nc.tensor.matmul(
                                py[:],
                                hT_sb[:, fb, cb * P : (cb + 1) * P],
                                w2_sb[e][:, fb, dc * 512 : (dc + 1) * 512],
                                start=(fb == 0),
                                stop=(fb == NFC - 1),
                            )
                        nc.vector.tensor_scalar_mul(
                            y_sb[:, dc * 512 : (dc + 1) * 512],
                            py[:],
                            cw_sb[:, blk : blk + 1],
                        )
                        if last_blk:
                            # final tail: per-half DMA so the last transfer
                            # (critical path to kernel end) is half as long
                            nc.sync.dma_start(
                                out=y[blk * P : (blk + 1) * P, dc * 512 : (dc + 1) * 512],
                                in_=y_sb[:, dc * 512 : (dc + 1) * 512],
                            )
                    if not last_blk:
                        nc.sync.dma_start(
                            out=y[blk * P : (blk + 1) * P, :],
                            in_=y_sb[:],
                        )
                blk0 += ncb
    nc.compile()
    return nc, sched


def _route(xf, router_w, router_b):
    logits = xf @ router_w + router_b
    logits = logits - logits.max(axis=1, keepdims=True)
    p = np.exp(logits)
    p /= p.sum(axis=1, keepdims=True)
    top_i = np.argsort(-p, axis=1, kind="stable")[:, :TOP_K]
    tp = np.take_along_axis(p, top_i, 1)
    tp = tp / tp.sum(axis=1, keepdims=True)
    return top_i, tp.astype(np.float32)


def kernel(x, w1, b1, w2, b2, router_w, router_b):
    x = np.asarray(x, np.float32)
    B, S, _ = x.shape
    T = B * S
    xf = x.reshape(T, D)
    w1f = np.asarray(w1, np.float32)
    w2f = np.asarray(w2, np.float32)
    b1f = np.asarray(b1, np.float32)
    b2f = np.asarray(b2, np.float32)

    top_i, tp = _route(xf, np.asarray(router_w, np.float32), np.asarray(router_b, np.float32))

    idxs, cws_l = [], []
    for e in range(E):
        sel = top_i == e
        rows = np.nonzero(sel.any(axis=1))[0]
        w = (tp * sel).sum(axis=1)[rows]
        idxs.append(rows)
        cws_l.append(w.astype(np.float32))

    caps = tuple(max(CC, ((len(r) + 127) // 128) * 128) for r in idxs)

    if caps not in _BUILD_CACHE:
        _BUILD_CACHE[caps] = _build(caps)
    nc, sched = _BUILD_CACHE[caps]

    NCH = len(sched)
    NBT = sum(caps) // P

    # xT: all experts' gathered tokens, flat chunk-contiguous — identical
    # for every core, so pack exactly once.
    XW = sum(ND * Cc for (_, Cc, _) in sched)
    xT = np.zeros((P, XW), BF16)
    cwf = np.zeros(NBT * P, np.float32)
    gcache = {}
    o = 0
    for ci, (e, Cc, off) in enumerate(sched):
        n = len(idxs[e])
        if e not in gcache and n:
            gcache[e] = (
                xf[idxs[e]].astype(BF16).T.reshape(ND, P, n).transpose(1, 0, 2)
            )  # [P, ND, n]
        take = min(max(n - off, 0), Cc)
        if take > 0:
            seg = np.zeros((P, ND, Cc), BF16)
            seg[:, :, :take] = gcache[e][:, :, off : off + take]
            xT[:, o : o + ND * Cc] = seg.reshape(P, ND * Cc)
        o += ND * Cc
    eoff = {}
    o = 0
    for e, C_e in enumerate(caps):
        eoff[e] = o
        cwf[o : o + len(cws_l[e])] = cws_l[e]
        o += C_e

    w1b = w1f.astype(BF16)
    w2b = w2f.astype(BF16)
    in_maps = []
    for j in range(E):
        fsl = slice(j * 512, (j + 1) * 512)
        # w1 f-slice: [E, D, 512] -> per-partition [E, ND, 512]
        w1j = np.ascontiguousarray(
            w1b[:, :, fsl].reshape(E, ND, P, 512).transpose(2, 0, 1, 3)
        )
        # w2 f-slice: [E, 512, D] -> per-partition [E, NFC, D]
        w2j = np.ascontiguousarray(
            w2b[:, fsl, :].reshape(E, NFC, P, D).transpose(2, 0, 1, 3)
        )
        b1j = np.ascontiguousarray(
            b1f[:, fsl].reshape(E, NFC, P).transpose(2, 0, 1)
        )
        in_maps.append(
            {
                "xTe": xT,
                "w1s": w1j,
                "w2s": w2j,
                "b1s": b1j,
                "cws": np.ascontiguousarray(cwf.reshape(NBT, P).T),
            }
        )

    res = run_bass_kernel_spmd(
        nc,
        in_maps,
        list(range(E)),
        trace=TRACE,
        trace_cores=list(range(E)) if TRACE_ALL else None,
    )
    LAST["exec_time_ns"] = res.exec_time_ns
    LAST["res"] = res
    LAST["C"] = caps

    ysum = np.zeros((NBT * P, D), np.float32)
    for j in range(E):
        ysum += np.asarray(res.results[j]["y"], np.float32)

    outf = np.zeros((T, D), np.float32)
    for e in range(E):
        n = len(idxs[e])
        if n:
            o = eoff[e]
            outf[idxs[e]] += ysum[o : o + n]
    cw_dense = np.zeros((T, E), np.float32)
    np.put_along_axis(cw_dense, top_i, tp, axis=1)
    outf += cw_dense @ b2f
    return outf.reshape(B, S, D)


# revision 7
# speedup vs baseline: 1.0820x; 1.0017x over previous
"""MoE FFN (top-2 of 8 experts) on 8 Trainium2 NeuronCores — v4.

Sharding: hidden-dimension (F) slicing instead of expert parallelism.
Every core processes ALL experts' routed tokens, but only a 512-wide slice
of the 4096 hidden units (w1[:, :, fsl], w2[:, fsl, :]). Host sums the 8
partial y contributions. This removes the max-expert-load imbalance that
expert parallelism pays under SPMD: per-core work is sum_e pad128(n_e)/8
token-blocks instead of max_e pad128(n_e), a ~3.7% FLOP reduction here.

Per core, for each expert e and each 512-token chunk of its tokens:
  hT = gelu(w1[e][:, fsl].T @ xT + b1[e][fsl])   # [512f, tok] in 4 fb tiles
  y_part = cw * (hT.T @ w2[e][fsl, :])           # [tok, 1024] bf16 out

DMA: weights + y output on the Sync ring; xT chunks (34MB, identical array
for every core) stream on the Scalar ring, triggers interleaved two chunks
ahead of consumption.
"""

import os
import sys

sys.path.insert(0, "/opt/trn_rl_repo")

import numpy as np
import ml_dtypes

import concourse.bass as bass
import concourse.bacc as bacc
import concourse.mybir as mybir
from concourse import tile
from concourse.bass_utils import run_bass_kernel_spmd

BF16 = ml_dtypes.bfloat16
P = 128
D, F, E = 1024, 4096, 8
ND = D // P  # 8
NFC = 4      # fb blocks per core (F/8 = 512 = 4 * 128)
TOP_K = 2

TRACE = bool(int(os.environ.get("MOE_TRACE", "0")))
TRACE_ALL = bool(int(os.environ.get("MOE_TRACE_ALL", "0")))
LAST = {}

_BUILD_CACHE = {}


def _enable_axon_profiling():
    import types

    if "antenv.axon_hooks" not in sys.modules:
        mod = types.ModuleType("antenv.axon_hooks")
        mod._hook = None

        def set_axon_ntff_profile_hook(h):
            mod._hook = h

        def get_axon_ntff_profile_hook():
            return mod._hook

        mod.set_axon_ntff_profile_hook = set_axon_ntff_profile_hook
        mod.get_axon_ntff_profile_hook = get_axon_ntff_profile_hook
        sys.modules["antenv.axon_hooks"] = mod
        import antenv

        antenv.axon_hooks = mod
    hooks = sys.modules["antenv.axon_hooks"]
    if hooks.get_axon_ntff_profile_hook() is None:
        from trn_agent_boot.trn_boot import _ntff_profile_via_ctypes

        hooks.set_axon_ntff_profile_hook(
            _ntff_profile_via_ctypes("/opt/axon/libaxon_pjrt.so")
        )
    import concourse.bass_utils as bu

    bu.upload_artifacts = lambda tmpdir: tmpdir


if TRACE:
    _enable_axon_profiling()


CC = 512
WARMUP_MM = 48


def _chunks_for(C):
    ch = [CC] * (C // CC)
    if C % CC:
        ch.append(C % CC)
    return ch


def _build(caps, counts):
    """caps: padded capacities (multiples of 128); counts: exact token counts.
    matmul-1 and the activation run on exact token counts (the moving axis
    needs no 128-padding); only matmul-2's token-partition blocks use the
    padded capacity. Padded tokens' y rows are cw=0-scaled garbage the host
    discards."""
    act_func = mybir.ActivationFunctionType.Gelu
    nc = bacc.Bacc()
    dt = mybir.dt

    # global chunk schedule: (expert, chunk_size, token_offset_in_expert).
    # The very first chunk is split to 256 tokens so the startup-critical
    # xT transfer is half as long.
    sched = []
    for e, C_e in enumerate(caps):
        off = 0
        if e == 0 and C_e >= CC:
            ch = [CC // 4, 3 * CC // 4] + _chunks_for(C_e - CC)
        else:
            ch = _chunks_for(C_e)
        for Cc in ch:
            sched.append((e, Cc, off))
            off += Cc
    NCH = len(sched)
    NBT = sum(caps) // P  # total 128-token blocks

    # flat variable-width xT: chunk ci occupies columns [xoff[ci], xoff[ci+1])
    xoff = [0]
    for (_, Cc, _) in sched:
        xoff.append(xoff[-1] + ND * Cc)
    XW = xoff[-1]

    xTe = nc.dram_tensor("xTe", [P, XW], dt.bfloat16, kind="ExternalInput")
    w1s = nc.dram_tensor("w1s", [P, E, ND, 512], dt.bfloat16, kind="ExternalInput")
    w2s = nc.dram_tensor("w2s", [P, E, NFC, D], dt.bfloat16, kind="ExternalInput")
    b1s = nc.dram_tensor("b1s", [P, E, NFC], dt.float32, kind="ExternalInput")
    cws = nc.dram_tensor("cws", [P, NBT], dt.float32, kind="ExternalInput")
    y = nc.dram_tensor("y", [NBT * P, D], dt.bfloat16, kind="ExternalOutput")

    with tile.TileContext(nc) as tc:
        with (
            tc.tile_pool(name="weights", bufs=1) as wpool,
            tc.tile_pool(name="consts", bufs=1) as cpool,
            tc.tile_pool(name="xin", bufs=5) as xpool,
            tc.tile_pool(name="hmid", bufs=2) as hpool,
            tc.tile_pool(name="yout", bufs=6) as ypool,
            tc.tile_pool(name="psh", bufs=4, space="PSUM") as psh,
            tc.tile_pool(name="psy", bufs=4, space="PSUM") as psy,
        ):
            # per-expert weight tiles; expert 0's w1 halved so the first
            # matmul waits on only 0.5MB
            w1_e0 = [wpool.tile([P, 4, 512], dt.bfloat16, name=f"w1_0{h}", tag=f"w1_0{h}") for h in range(2)]
            w1_sb = [wpool.tile([P, ND, 512], dt.bfloat16, name=f"w1_{e}", tag=f"w1_{e}") for e in range(1, E)]
            w2_sb = [wpool.tile([P, NFC, D], dt.bfloat16, name=f"w2_{e}", tag=f"w2_{e}") for e in range(E)]
            b1_sb = cpool.tile([P, E, NFC], dt.float32)
            cw_sb = cpool.tile([P, NBT], dt.float32)

            def w1_slice(e, fb, kd):
                if e == 0:
                    return w1_e0[kd // 4][:, kd % 4, fb * P : (fb + 1) * P]
                return w1_sb[e - 1][:, kd, fb * P : (fb + 1) * P]

            xT_sb = [
                xpool.tile([P, ND * sched[c][1]], dt.bfloat16, name=f"xT_{c}", tag="xT")
                for c in range(NCH)
            ]

            def issue_x(c):
                nc.scalar.dma_start(
                    out=xT_sb[c][:], in_=xTe[:, xoff[c] : xoff[c + 1]]
                )

            # chunks 0-1 now; later chunks are triggered from inside the
            # activation stream two chunks ahead
            issue_x(0)
            issue_x(1)

            # Upfront only what the first two experts need (~4MB), ordered
            # by need time — the rest of w1/w2 is issued just-in-time from
            # inside the compute stream so the y output DMAs (same Sync
            # ring, strict FIFO) never queue behind bulk weight traffic.
            nc.sync.dma_start(out=w1_e0[0][:], in_=w1s[:, 0, 0:4, :])
            nc.sync.dma_start(out=w1_e0[1][:], in_=w1s[:, 0, 4:8, :])
            nc.sync.dma_start(out=b1_sb[:], in_=b1s[:])
            nc.sync.dma_start(out=w1_sb[0][:], in_=w1s[:, 1])
            nc.sync.dma_start(out=w2_sb[0][:], in_=w2s[:, 0])
            nc.sync.dma_start(out=cw_sb[:], in_=cws[:])
            nc.sync.dma_start(out=w2_sb[1][:], in_=w2s[:, 1])

            def issue_expert_weights(e):
                if e < E:
                    nc.sync.dma_start(out=w1_sb[e - 1][:], in_=w1s[:, e])
                    nc.sync.dma_start(out=w2_sb[e][:], in_=w2s[:, e])

            warm_l = cpool.tile([P, P], dt.bfloat16)
            nc.vector.memset(warm_l[:], 0.0)
            warm_ps = psy.tile([P, 512], dt.float32, tag="py")
            for i in range(WARMUP_MM):
                nc.tensor.matmul(
                    warm_ps[:, :P], warm_l[:], warm_l[:],
                    start=(i == 0), stop=(i == WARMUP_MM - 1),
                )

            blk0 = 0
            issued = 1
            first_chunk_of = {}
            for ci, (e, Cc, off) in enumerate(sched):
                first_chunk_of.setdefault(e, ci)
            for ci, (e, Cc, off) in enumerate(sched):
                ncb = Cc // P
                take = min(counts[e] - off, Cc)
                hT_sb = hpool.tile([P, NFC, CC], dt.bfloat16, tag="hT")
                for fb in range(NFC):
                    ph = psh.tile([P, CC], dt.float32, tag="ph")
                    for kd in range(ND):
                        nc.tensor.matmul(
                            ph[:, :take],
                            w1_slice(e, fb, kd),
                            xT_sb[ci][:, kd * Cc : kd * Cc + take],
                            start=(kd == 0),
                            stop=(kd == ND - 1),
                        )
                    nc.scalar.activation(
                        hT_sb[:, fb, :take],
                        ph[:, :take],
                        act_func,
                        bias=b1_sb[:, e, fb : fb + 1],
                    )
                    # stream the x chunks two ahead of consumption; trigger
                    # only after the last activation of the chunk so the
                    # trigger's engine time never delays an activation that
                    # gates matmul-2's short accumulation chains
                    if fb == NFC - 1:
                        while issued < min(ci + 2, NCH - 1):
                            issued += 1
                            issue_x(issued)
                        if first_chunk_of.get(e) == ci:
                            issue_expert_weights(e + 2)
                for cb in range(ncb):
                    y_sb = ypool.tile([P, D], dt.bfloat16, tag="y")
                    blk = blk0 + cb
                    last_blk = ci == NCH - 1 and cb == ncb - 1
                    for dc in range(2):
                        py = psy.tile([P, 512], dt.float32, tag="py")
                        for fb in range(NFC):
                            nc.tensor.matmul(
                                py[:],
                                hT_sb[:, fb, cb * P : (cb + 1) * P],
                                w2_sb[e][:, fb, dc * 512 : (dc + 1) * 512],
                                start=(fb == 0),
                                stop=(fb == NFC - 1),
                            )
                        nc.vector.tensor_scalar_mul(
                            y_sb[:, dc * 512 : (dc + 1) * 512],
                            py[:],
                            cw_sb[:, blk : blk + 1],
                        )
                        if last_blk:
                            # final tail: per-half DMA so the last transfer
                            # (critical path to kernel end) is half as long
                            nc.sync.dma_start(
                                out=y[blk * P : (blk + 1) * P, dc * 512 : (dc + 1) * 512],
                                in_=y_sb[:, dc * 512 : (dc + 1) * 512],
                            )
                    if not last_blk:
                        nc.sync.dma_start(
                            out=y[blk * P : (blk + 1) * P, :],
                            in_=y_sb[:],
                        )
                blk0 += ncb
    nc.compile()
    return nc, sched


def _route(xf, router_w, router_b):
    logits = xf @ router_w + router_b
    logits = logits - logits.max(axis=1, keepdims=True)
    p = np.exp(logits)
    p /= p.sum(axis=1, keepdims=True)
    top_i = np.argsort(-p, axis=1, kind="stable")[:, :TOP_K]
    tp = np.take_along_axis(p, top_i, 1)
    tp = tp / tp.sum(axis=1, keepdims=True)
    return top_i, tp.astype(np.float32)


def kernel(x, w1, b1, w2, b2, router_w, router_b):
    x = np.asarray(x, np.float32)
    B, S, _ = x.shape
    T = B * S
    xf = x.reshape(T, D)
    w1f = np.asarray(w1, np.float32)
    w2f = np.asarray(w2, np.float32)
    b1f = np.asarray(b1, np.float32)
    b2f = np.asarray(b2, np.float32)

    top_i, tp = _route(xf, np.asarray(router_w, np.float32), np.asarray(router_b, np.float32))

    idxs, cws_l = [], []
    for e in range(E):
        sel = top_i == e
        rows = np.nonzero(sel.any(axis=1))[0]
        w = (tp * sel).sum(axis=1)[rows]
        idxs.append(rows)
        cws_l.append(w.astype(np.float32))

    caps = tuple(max(CC, ((len(r) + 127) // 128) * 128) for r in idxs)
    counts = tuple(len(r) for r in idxs)

    if counts not in _BUILD_CACHE:
        _BUILD_CACHE[counts] = _build(caps, counts)
    nc, sched = _BUILD_CACHE[counts]

    NCH = len(sched)
    NBT = sum(caps) // P

    # xT: all experts' gathered tokens, flat chunk-contiguous — identical
    # for every core, so pack exactly once.
    XW = sum(ND * Cc for (_, Cc, _) in sched)
    xT = np.zeros((P, XW), BF16)
    cwf = np.zeros(NBT * P, np.float32)
    gcache = {}
    o = 0
    for ci, (e, Cc, off) in enumerate(sched):
        n = len(idxs[e])
        if e not in gcache and n:
            gcache[e] = (
                xf[idxs[e]].astype(BF16).T.reshape(ND, P, n).transpose(1, 0, 2)
            )  # [P, ND, n]
        take = min(max(n - off, 0), Cc)
        if take > 0:
            seg = np.zeros((P, ND, Cc), BF16)
            seg[:, :, :take] = gcache[e][:, :, off : off + take]
            xT[:, o : o + ND * Cc] = seg.reshape(P, ND * Cc)
        o += ND * Cc
    eoff = {}
    o = 0
    for e, C_e in enumerate(caps):
        eoff[e] = o
        cwf[o : o + len(cws_l[e])] = cws_l[e]
        o += C_e

    w1b = w1f.astype(BF16)
    w2b = w2f.astype(BF16)
    in_maps = []
    for j in range(E):
        fsl = slice(j * 512, (j + 1) * 512)
        # w1 f-slice: [E, D, 512] -> per-partition [E, ND, 512]
        w1j = np.ascontiguousarray(
            w1b[:, :, fsl].reshape(E, ND, P, 512).transpose(2, 0, 1, 3)
        )
        # w2 f-slice: [E, 512, D] -> per-partition [E, NFC, D]
        w2j = np.ascontiguousarray(
            w2b[:, fsl, :].reshape(E, NFC, P, D).transpose(2, 0, 1, 3)
        )
        b1j = np.ascontiguousarray(
            b1f[:, fsl].reshape(E, NFC, P).transpose(2, 0, 1)
        )
        in_maps.append(
            {
                "xTe": xT,
                "w1s": w1j,
                "w2s": w2j,
                "b1s": b1j,
                "cws": np.ascontiguousarray(cwf.reshape(NBT, P).T),
            }
        )

    res = run_bass_kernel_spmd(
        nc,
        in_maps,
        list(range(E)),
        trace=TRACE,
        trace_cores=list(range(E)) if TRACE_ALL else None,
    )
    LAST["exec_time_ns"] = res.exec_time_ns
    LAST["res"] = res
    LAST["C"] = caps

    ysum = np.zeros((NBT * P, D), np.float32)
    for j in range(E):
        ysum += np.asarray(res.results[j]["y"], np.float32)

    outf = np.zeros((T, D), np.float32)
    for e in range(E):
        n = len(idxs[e])
        if n:
            o = eoff[e]
            outf[idxs[e]] += ysum[o : o + n]
    cw_dense = np.zeros((T, E), np.float32)
    np.put_along_axis(cw_dense, top_i, tp, axis=1)
    outf += cw_dense @ b2f
    return outf.reshape(B, S, D)
